# revision 5
# baseline (speedup 1.0000x reference)
"""DifferentialCausalAttention on 8 Trainium2 NeuronCores.

Sharding: 8 cores = 2 batches x 4 head-groups (tensor-parallel over heads).
Core c handles batch b = c // 4 and head-group g = c % 4:
  - query heads 8g..8g+7 (4 pairs), kv heads 4g..4g+3, lambda cols 4g..4g+3
  - W_O rows 512g..512g+511 -> partial output, host-summed over the 4 groups.

All matmuls run in float32r (full-rate fp32 mode on the PE).
Layouts on device: Q^T/K^T as [dh, L] (dh on partitions), V as [L, d],
attention computed transposed (S^T = [k, q]) so no P-transposes are needed.
"""
from contextlib import ExitStack

import numpy as np

import concourse.bass as bass
import concourse.mybir as mybir
import concourse.tile as tile
from concourse import bacc
from concourse.bass_utils import run_bass_kernel_spmd

F32 = mybir.dt.float32
F32R = mybir.dt.float32r

B, L, D, NH = 2, 2048, 2048, 16
DH = D // NH            # 128
G = 4                   # head groups (cores per batch)
NKV = NH // G           # kv heads per core = 4
NQ = 2 * NKV            # query heads per core = 8
CQK = NQ * DH + NKV * DH  # 1536 projection cols (Q then K)
CT = CQK // 128         # 12 column tiles (0-7 Q heads, 8-11 K heads)
DC = D // 128           # 16 contraction chunks
LCH = L // 512          # 4 L-chunks
LT = L // 128           # 16 L-tiles / q-tiles
SCALE = 1.0 / float(np.sqrt(DH))
ROPE_BASE = 10000.0


def build_kernel() -> bacc.Bacc:
    nc = bacc.Bacc("TRN2", target_bir_lowering=False, debug=False)

    xT = nc.dram_tensor("xT", [D, L], F32R, kind="ExternalInput")
    Wqk = nc.dram_tensor("Wqk", [D, CQK], F32R, kind="ExternalInput")
    Wv = nc.dram_tensor("Wv", [D, NKV * DH], F32R, kind="ExternalInput")
    Wl = nc.dram_tensor("Wl", [D, NKV], F32R, kind="ExternalInput")
    blv = nc.dram_tensor("blv", [NKV, 1], F32, kind="ExternalInput")
    Wo = nc.dram_tensor("Wo", [NKV * DH, D], F32R, kind="ExternalInput")
    cosT = nc.dram_tensor("cosT", [DH, L], F32, kind="ExternalInput")
    sinTs = nc.dram_tensor("sinTs", [DH, L], F32, kind="ExternalInput")
    maskT = nc.dram_tensor("maskT", [128, 256], F32R, kind="ExternalInput")
    onesin = nc.dram_tensor("onesin", [128, 128], F32R, kind="ExternalInput")
    outT = nc.dram_tensor("outT", [D, L], F32, kind="ExternalOutput")

    with ExitStack() as ctx:
        tc = ctx.enter_context(tile.TileContext(nc))

        persist = ctx.enter_context(tc.tile_pool(name="persist", bufs=1))
        dram = ctx.enter_context(tc.tile_pool(name="dram", bufs=1, space="DRAM"))
        psum = ctx.enter_context(tc.tile_pool(name="psum", bufs=1, space="PSUM"))

        # ---- persistent tiles ----
        mask_sb = persist.tile([128, 256], F32R)
        nc.sync.dma_start(mask_sb[:], maskT[:, :])
        ones_sb = persist.tile([128, 128], F32R)
        nc.sync.dma_start(ones_sb[:], onesin[:, :])
        bl_sb = persist.tile([NKV, 1], F32)
        nc.sync.dma_start(bl_sb[:], blv[:, :])
        lam_sb = persist.tile([NKV, L], F32)          # sigmoid(x@Wl+bl), row per kv head
        diffT = persist.tile([128, NKV, L], F32R)     # (ctx0 - lam*ctx1)^T per head

        # DRAM scratch between phases
        qkT_d = dram.tile([CT, 128, L], F32R)         # Q^T/K^T after RoPE
        v_d = dram.tile([L, NKV * DH], F32R)          # V in [L, d] layout

        # ================= Phase 1: projections + RoPE =================
        with tc.tile_pool(name="ph1", bufs=1) as ph1:
            wv_sb = ph1.tile([128, DC, NKV * DH], F32R)
            nc.sync.dma_start(wv_sb[:], Wv.rearrange("(dc p) c -> p dc c", p=128))
            wl_sb = ph1.tile([128, DC, NKV], F32R)
            nc.sync.dma_start(wl_sb[:], Wl.rearrange("(dc p) c -> p dc c", p=128))
            cos_sb = ph1.tile([128, L], F32)
            nc.sync.dma_start(cos_sb[:], cosT[:, :])
            sin_sb = ph1.tile([128, L], F32)
            nc.sync.dma_start(sin_sb[:], sinTs[:, :])

            xTr = xT.rearrange("(dc p) l -> p dc l", p=128)
            wqkr = Wqk.rearrange("(dc p) c -> p dc c", p=128)

            for lch in range(LCH):
                ls = slice(lch * 512, (lch + 1) * 512)
                xs = ph1.tile([128, DC, 512], F32R, name="xs", tag="xs", bufs=2)
                nc.sync.dma_start(xs[:], xTr[:, :, ls])

                # --- Q^T / K^T column tiles + RoPE ---
                for ct in range(CT):
                    wt = ph1.tile([128, DC, 128], F32R, name="wt", tag="wt", bufs=3)
                    nc.sync.dma_start(wt[:], wqkr[:, :, ct * 128:(ct + 1) * 128])
                    qk_ps = psum.tile([128, 512], F32, name="qk_ps", tag="mm512", bufs=2)
                    for dc in range(DC):
                        nc.tensor.matmul(
                            qk_ps[:], wt[:, dc, :], xs[:, dc, :],
                            start=(dc == 0), stop=(dc == DC - 1),
                        )
                    # RoPE: qr = qk*cos + rot(qk)*sin_signed
                    rot = ph1.tile([128, 512], F32, name="rot", tag="rot", bufs=2)
                    nc.scalar.copy(rot[0:64, :], qk_ps[64:128, :])
                    nc.scalar.copy(rot[64:128, :], qk_ps[0:64, :])
                    t1 = ph1.tile([128, 512], F32, name="t1", tag="t1", bufs=2)
                    nc.vector.tensor_mul(t1[:], qk_ps[:], cos_sb[:, ls])
                    t2 = ph1.tile([128, 512], F32, name="t2", tag="t2", bufs=2)
                    nc.vector.tensor_mul(t2[:], rot[:], sin_sb[:, ls])
                    qkr_sb = ph1.tile([128, 512], F32R, name="qkr_sb", tag="qkr", bufs=3)
                    nc.vector.tensor_add(qkr_sb[:], t1[:], t2[:])
                    nc.sync.dma_start(qkT_d[ct, :, ls], qkr_sb[:])

                # --- V tiles ---
                for lt in range(4):
                    v_ps = psum.tile([128, 512], F32, name="v_ps", tag="mm512", bufs=2)
                    for dc in range(DC):
                        nc.tensor.matmul(
                            v_ps[:], xs[:, dc, lt * 128:(lt + 1) * 128], wv_sb[:, dc, :],
                            start=(dc == 0), stop=(dc == DC - 1),
                        )
                    v_sb = ph1.tile([128, 512], F32R, name="v_sb", tag="v_sb", bufs=2)
                    nc.scalar.copy(v_sb[:], v_ps[:])
                    nc.sync.dma_start(
                        v_d[lch * 512 + lt * 128: lch * 512 + (lt + 1) * 128, :], v_sb[:]
                    )

                # --- lambda ---
                lam_ps = psum.tile([NKV, 512], F32, name="lam_ps", tag="small", bufs=1)
                for dc in range(DC):
                    nc.tensor.matmul(
                        lam_ps[:], wl_sb[:, dc, :], xs[:, dc, :],
                        start=(dc == 0), stop=(dc == DC - 1),
                    )
                nc.scalar.activation(
                    lam_sb[:, ls], lam_ps[:],
                    mybir.ActivationFunctionType.Sigmoid, bias=bl_sb[:, 0:1],
                )

        # ================= Phase 2: causal attention per head pair =================
        with tc.tile_pool(name="ph2", bufs=1) as ph2:
            v_r = v_d.rearrange("(kc pp) d -> pp kc d", pp=128)
            for p in range(NKV):
                qt_sb = ph2.tile([128, 2, L], F32R, name="qt_sb", tag="qt", bufs=2)
                nc.sync.dma_start(
                    qt_sb[:], qkT_d[2 * p:2 * p + 2, :, :].rearrange("h p l -> p h l")
                )
                lam0 = ph2.tile([1, L], F32, name="lam0", tag="lam0", bufs=2)
                nc.sync.dma_start(lam0[:], lam_sb[p:p + 1, :])
                kt_sb = ph2.tile([128, L], F32R, name="kt_sb", tag="kt", bufs=2)
                nc.sync.dma_start(kt_sb[:], qkT_d[NQ + p, :, :])
                vp_sb = ph2.tile([128, LT, 128], F32R, name="vp_sb", tag="vp", bufs=2)
                nc.sync.dma_start(vp_sb[:], v_r[:, :, p * 128:(p + 1) * 128])

                for qt in range(LT):
                    ctx_ps = psum.tile([128, 256], F32, name="ctx_ps", tag="ctx", bufs=2)
                    rs_ps = psum.tile([1, 256], F32, name="rs_ps", tag="small", bufs=1)
                    pend = None  # (e_tile, kc) waiting for its ctx/rowsum matmuls
                    for kc in range(qt + 1):
                        s_ps = psum.tile([128, 256], F32, name="s_ps", tag="s", bufs=3)
                        nc.tensor.matmul(
                            s_ps[:],
                            kt_sb[:, kc * 128:(kc + 1) * 128],
                            qt_sb[:, :, qt * 128:(qt + 1) * 128],
                            start=True, stop=True,
                        )
                        e_sb = ph2.tile([128, 256], F32R, name="e_sb", tag="e", bufs=3)
                        nc.scalar.activation(
                            e_sb[:], s_ps[:], mybir.ActivationFunctionType.Exp,
                            scale=SCALE,
                        )
                        if kc == qt:
                            nc.vector.tensor_mul(e_sb[:], e_sb[:], mask_sb[:])
                        if pend is not None:
                            _emit_ctx(nc, ctx_ps, rs_ps, vp_sb, ones_sb, pend, qt)
                        pend = (e_sb, kc)
                    _emit_ctx(nc, ctx_ps, rs_ps, vp_sb, ones_sb, pend, qt)

                    # normalization + lambda as per-column scales
                    recip = ph2.tile([1, 256], F32, name="recip", tag="recip", bufs=2)
                    nc.vector.reciprocal(recip[:], rs_ps[:])
                    cs = ph2.tile([1, 256], F32R, name="cs", tag="cs", bufs=2)
                    nc.vector.tensor_copy(cs[:, 0:128], recip[:, 0:128])
                    nc.vector.tensor_mul(
                        cs[:, 128:256], recip[:, 128:256],
                        lam0[:, qt * 128:(qt + 1) * 128],
                    )
                    b_ps = psum.tile([128, 256], F32, name="b_ps", tag="s", bufs=3)
                    nc.tensor.matmul(
                        b_ps[:], ones_sb[0:1, :], cs[0:1, :], start=True, stop=True,
                    )
                    b_sb = ph2.tile([128, 256], F32, name="b_sb", tag="bsb", bufs=2)
                    nc.scalar.copy(b_sb[:], b_ps[:])
                    t0 = ph2.tile([128, 128], F32, name="t0", tag="t0", bufs=2)
                    nc.vector.tensor_mul(t0[:], ctx_ps[:, 0:128], b_sb[:, 0:128])
                    t1b = ph2.tile([128, 128], F32, name="t1b", tag="t1b", bufs=2)
                    nc.vector.tensor_mul(t1b[:], ctx_ps[:, 128:256], b_sb[:, 128:256])
                    nc.vector.tensor_sub(
                        diffT[:, p, qt * 128:(qt + 1) * 128], t0[:], t1b[:]
                    )

        # ================= Phase 3: output projection =================
        with tc.tile_pool(name="ph3", bufs=1) as ph3:
            wo_sb = ph3.tile([128, NKV, D], F32R)
            nc.sync.dma_start(wo_sb[:], Wo.rearrange("(p d) o -> d p o", d=128))
            for ot in range(LT):
                for qch in range(LCH):
                    o_ps = psum.tile([128, 512], F32, name="o_ps", tag="mm512", bufs=2)
                    for p in range(NKV):
                        nc.tensor.matmul(
                            o_ps[:],
                            wo_sb[:, p, ot * 128:(ot + 1) * 128],
                            diffT[:, p, qch * 512:(qch + 1) * 512],
                            start=(p == 0), stop=(p == NKV - 1),
                        )
                    o_sb = ph3.tile([128, 512], F32, name="o_sb", tag="osb", bufs=3)
                    nc.scalar.copy(o_sb[:], o_ps[:])
                    nc.sync.dma_start(
                        outT[ot * 128:(ot + 1) * 128, qch * 512:(qch + 1) * 512], o_sb[:]
                    )

    nc.finalize()
    return nc


def _emit_ctx(nc, ctx_ps, rs_ps, vp_sb, ones_sb, pend, qt):
    e_sb, kc = pend
    nc.tensor.matmul(
        ctx_ps[:], vp_sb[:, kc, :], e_sb[:],
        start=(kc == 0), stop=(kc == qt), skip_group_check=True,
    )
    nc.tensor.matmul(
        rs_ps[:], ones_sb[:, 0:1], e_sb[:],
        start=(kc == 0), stop=(kc == qt), skip_group_check=True,
    )


def _host_tables():
    half = DH // 2
    inv_freq = 1.0 / (ROPE_BASE ** (np.arange(0, half, dtype=np.float64) * 2.0 / DH))
    freqs = np.arange(L, dtype=np.float64)[:, None] * inv_freq[None, :]  # [L, half]
    emb = np.concatenate([freqs, freqs], axis=-1)  # [L, DH]
    cosT = np.ascontiguousarray(np.cos(emb).T.astype(np.float32))  # [DH, L]
    sinT = np.sin(emb).T.astype(np.float32)
    sinTs = np.concatenate([-sinT[:half], sinT[half:]], axis=0)
    sinTs = np.ascontiguousarray(sinTs.astype(np.float32))
    tri = np.triu(np.ones((128, 128), dtype=np.float32))  # keep k' <= q'
    maskT = np.ascontiguousarray(np.concatenate([tri, tri], axis=1))
    ones = np.ones((128, 128), dtype=np.float32)
    return cosT, sinTs, maskT, ones


_NC_CACHE = []


def kernel(x, Wq, Wk, Wv, Wl, bl, Wo):
    x = np.asarray(x, dtype=np.float32)
    Wq = np.asarray(Wq, dtype=np.float32)
    Wk = np.asarray(Wk, dtype=np.float32)
    Wv = np.asarray(Wv, dtype=np.float32)
    Wl = np.asarray(Wl, dtype=np.float32)
    bl = np.asarray(bl, dtype=np.float32)
    Wo = np.asarray(Wo, dtype=np.float32)

    cosT, sinTs, maskT, ones = _host_tables()
    Wq3 = Wq.reshape(D, 2 * NH, DH)
    Wk3 = Wk.reshape(D, NH, DH)

    in_maps = []
    for c in range(8):
        b, g = divmod(c, G)
        wq_s = Wq3[:, 8 * g:8 * g + NQ, :].reshape(D, NQ * DH)
        wk_s = Wk3[:, G * g:G * g + NKV, :].reshape(D, NKV * DH)
        in_maps.append({
            "xT": np.ascontiguousarray(x[b].T),
            "Wqk": np.ascontiguousarray(np.concatenate([wq_s, wk_s], axis=1)),
            "Wv": np.ascontiguousarray(Wv[:, DH * G * g:DH * G * g + NKV * DH]),
            "Wl": np.ascontiguousarray(Wl[:, G * g:G * g + NKV]),
            "blv": np.ascontiguousarray(bl[G * g:G * g + NKV].reshape(NKV, 1)),
            "Wo": np.ascontiguousarray(Wo[512 * g:512 * (g + 1), :]),
            "cosT": cosT,
            "sinTs": sinTs,
            "maskT": maskT,
            "onesin": ones,
        })

    if not _NC_CACHE:
        _NC_CACHE.append(build_kernel())
    nc = _NC_CACHE[0]
    res = run_bass_kernel_spmd(nc, in_maps, core_ids=list(range(8)))

    out = np.empty((B, L, D), dtype=np.float32)
    for b in range(B):
        acc = res.results[4 * b]["outT"].copy()
        for g in range(1, G):
            acc += res.results[4 * b + g]["outT"]
        out[b] = acc.T
    return out


# revision 7
# speedup vs baseline: 1.0665x; 1.0665x over previous
"""DifferentialCausalAttention on 8 Trainium2 NeuronCores.

Sharding: 8 cores = 2 batches x 4 head-groups (tensor-parallel over heads).
Core c handles batch b = c // 4 and head-group g = c % 4:
  - query heads 8g..8g+7 (4 pairs), kv heads 4g..4g+3, lambda cols 4g..4g+3
  - W_O rows 512g..512g+511 -> partial output, host-summed over the 4 groups.

All matmuls run in float32r (full-rate fp32 mode on the PE).
Layouts on device: Q^T/K^T as [dh, L] (dh on partitions), V as [L, d],
attention computed transposed (S^T = [k, q]) so no P-transposes are needed.
"""
from contextlib import ExitStack

import numpy as np

import concourse.bass as bass
import concourse.mybir as mybir
import concourse.tile as tile
from concourse import bacc
from concourse.bass_utils import run_bass_kernel_spmd

F32 = mybir.dt.float32
F32R = mybir.dt.float32r

B, L, D, NH = 2, 2048, 2048, 16
DH = D // NH            # 128
G = 4                   # head groups (cores per batch)
NKV = NH // G           # kv heads per core = 4
NQ = 2 * NKV            # query heads per core = 8
CQK = NQ * DH + NKV * DH  # 1536 projection cols (Q then K)
CT = CQK // 128         # 12 column tiles (0-7 Q heads, 8-11 K heads)
DC = D // 128           # 16 contraction chunks
LCH = L // 512          # 4 L-chunks
LT = L // 128           # 16 L-tiles / q-tiles
SCALE = 1.0 / float(np.sqrt(DH))
ROPE_BASE = 10000.0


def build_kernel() -> bacc.Bacc:
    nc = bacc.Bacc("TRN2", target_bir_lowering=False, debug=False)

    xT = nc.dram_tensor("xT", [D, L], F32R, kind="ExternalInput")
    Wqk = nc.dram_tensor("Wqk", [D, CQK], F32R, kind="ExternalInput")
    Wv = nc.dram_tensor("Wv", [D, NKV * DH], F32R, kind="ExternalInput")
    Wl = nc.dram_tensor("Wl", [D, NKV], F32R, kind="ExternalInput")
    blv = nc.dram_tensor("blv", [NKV, 1], F32, kind="ExternalInput")
    Wo = nc.dram_tensor("Wo", [NKV * DH, D], F32R, kind="ExternalInput")
    cosT = nc.dram_tensor("cosT", [DH, L], F32, kind="ExternalInput")
    sinTs = nc.dram_tensor("sinTs", [DH, L], F32, kind="ExternalInput")
    maskT = nc.dram_tensor("maskT", [128, 256], F32R, kind="ExternalInput")
    onesin = nc.dram_tensor("onesin", [128, 128], F32R, kind="ExternalInput")
    outT = nc.dram_tensor("outT", [D, L], F32, kind="ExternalOutput")

    with ExitStack() as ctx:
        tc = ctx.enter_context(tile.TileContext(nc))

        persist = ctx.enter_context(tc.tile_pool(name="persist", bufs=1))
        dram = ctx.enter_context(tc.tile_pool(name="dram", bufs=1, space="DRAM"))
        psum = ctx.enter_context(tc.tile_pool(name="psum", bufs=1, space="PSUM"))

        # ---- persistent tiles ----
        mask_sb = persist.tile([128, 256], F32R)
        nc.sync.dma_start(mask_sb[:], maskT[:, :])
        ones_sb = persist.tile([128, 128], F32R)
        nc.sync.dma_start(ones_sb[:], onesin[:, :])
        bl_sb = persist.tile([NKV, 1], F32)
        nc.sync.dma_start(bl_sb[:], blv[:, :])
        lam_sb = persist.tile([NKV, L], F32)          # sigmoid(x@Wl+bl), row per kv head
        diffT = persist.tile([128, NKV, L], F32R)     # (ctx0 - lam*ctx1)^T per head

        # DRAM scratch between phases
        qkT_d = dram.tile([CT, 128, L], F32R)         # Q^T/K^T after RoPE
        v_d = dram.tile([L, NKV * DH], F32R)          # V in [L, d] layout

        # ================= Phase 1: projections + RoPE =================
        with tc.tile_pool(name="ph1", bufs=1) as ph1:
            wv_sb = ph1.tile([128, DC, NKV * DH], F32R)
            nc.sync.dma_start(wv_sb[:], Wv.rearrange("(dc p) c -> p dc c", p=128))
            wl_sb = ph1.tile([128, DC, NKV], F32R)
            nc.sync.dma_start(wl_sb[:], Wl.rearrange("(dc p) c -> p dc c", p=128))
            cos_sb = ph1.tile([128, L], F32)
            nc.sync.dma_start(cos_sb[:], cosT[:, :])
            sin_sb = ph1.tile([128, L], F32)
            nc.sync.dma_start(sin_sb[:], sinTs[:, :])

            xTr = xT.rearrange("(dc p) l -> p dc l", p=128)
            wqkr = Wqk.rearrange("(dc p) c -> p dc c", p=128)

            for lch in range(LCH):
                ls = slice(lch * 512, (lch + 1) * 512)
                xs = ph1.tile([128, DC, 512], F32R, name="xs", tag="xs", bufs=2)
                nc.sync.dma_start(xs[:], xTr[:, :, ls])

                # --- Q^T / K^T column tiles + RoPE ---
                for ct in range(CT):
                    wt = ph1.tile([128, DC, 128], F32R, name="wt", tag="wt", bufs=3)
                    nc.sync.dma_start(wt[:], wqkr[:, :, ct * 128:(ct + 1) * 128])
                    qk_ps = psum.tile([128, 512], F32, name="qk_ps", tag="mm512", bufs=2)
                    for dc in range(DC):
                        nc.tensor.matmul(
                            qk_ps[:], wt[:, dc, :], xs[:, dc, :],
                            start=(dc == 0), stop=(dc == DC - 1),
                        )
                    # RoPE: qr = qk*cos + rot(qk)*sin_signed
                    rot = ph1.tile([128, 512], F32, name="rot", tag="rot", bufs=2)
                    nc.scalar.copy(rot[0:64, :], qk_ps[64:128, :])
                    nc.scalar.copy(rot[64:128, :], qk_ps[0:64, :])
                    t1 = ph1.tile([128, 512], F32, name="t1", tag="t1", bufs=2)
                    nc.vector.tensor_mul(t1[:], qk_ps[:], cos_sb[:, ls])
                    t2 = ph1.tile([128, 512], F32, name="t2", tag="t2", bufs=2)
                    nc.vector.tensor_mul(t2[:], rot[:], sin_sb[:, ls])
                    qkr_sb = ph1.tile([128, 512], F32R, name="qkr_sb", tag="qkr", bufs=3)
                    nc.vector.tensor_add(qkr_sb[:], t1[:], t2[:])
                    nc.sync.dma_start(qkT_d[ct, :, ls], qkr_sb[:])

                # --- V tiles ---
                for lt in range(4):
                    v_ps = psum.tile([128, 512], F32, name="v_ps", tag="mm512", bufs=2)
                    for dc in range(DC):
                        nc.tensor.matmul(
                            v_ps[:], xs[:, dc, lt * 128:(lt + 1) * 128], wv_sb[:, dc, :],
                            start=(dc == 0), stop=(dc == DC - 1),
                        )
                    v_sb = ph1.tile([128, 512], F32R, name="v_sb", tag="v_sb", bufs=2)
                    nc.scalar.copy(v_sb[:], v_ps[:])
                    nc.sync.dma_start(
                        v_d[lch * 512 + lt * 128: lch * 512 + (lt + 1) * 128, :], v_sb[:]
                    )

                # --- lambda ---
                lam_ps = psum.tile([NKV, 512], F32, name="lam_ps", tag="small", bufs=1)
                for dc in range(DC):
                    nc.tensor.matmul(
                        lam_ps[:], wl_sb[:, dc, :], xs[:, dc, :],
                        start=(dc == 0), stop=(dc == DC - 1),
                    )
                nc.scalar.activation(
                    lam_sb[:, ls], lam_ps[:],
                    mybir.ActivationFunctionType.Sigmoid, bias=bl_sb[:, 0:1],
                )

        # ================= Phase 2: causal attention per head pair =================
        # Chunks (128 k-positions) are processed in blocks of 2 (one [128,512]
        # PSUM bank): per block 1-2 S matmuls, one exp, one rowsum matmul,
        # per-chunk ctx matmuls. ctx/rs for block b are emitted after the S
        # matmuls of block b+2 so the PE never waits on ACT. The per-qtile
        # normalization tail is deferred into the next qtile (pend_norm).
        with tc.tile_pool(name="ph2", bufs=1) as ph2:
            v_r = v_d.rearrange("(kc pp) d -> pp kc d", pp=128)
            pend_norm = [None]

            def emit_block(st):
                ctx_ps, rs_ps, e_sb, kcs, qt, wid = st
                for j, kc in enumerate(kcs):
                    nc.tensor.matmul(
                        ctx_ps[:], vp_sb[:, kc, :], e_sb[:, j * 256:(j + 1) * 256],
                        start=(kc == 0), stop=(kc == qt), skip_group_check=True,
                    )
                nc.tensor.matmul(
                    rs_ps[0:1, 0:wid], ones_sb[:, 0:1], e_sb[:, 0:wid],
                    start=(kcs[0] == 0), stop=(kcs[-1] == qt), skip_group_check=True,
                )

            def emit_norm():
                st = pend_norm[0]
                if st is None:
                    return
                pend_norm[0] = None
                ctx_ps, rs_ps, qt, p_, lam0_ = st
                rs_sb = ph2.tile([1, 512], F32, name="rs_sb", tag="rs_sb", bufs=2)
                nc.scalar.copy(rs_sb[:], rs_ps[:])
                den = ph2.tile([1, 256], F32, name="den", tag="den", bufs=2)
                if qt == 0:
                    nc.vector.tensor_copy(den[:], rs_sb[:, 0:256])
                else:
                    nc.vector.tensor_add(den[:], rs_sb[:, 0:256], rs_sb[:, 256:512])
                recip = ph2.tile([1, 256], F32, name="recip", tag="recip", bufs=2)
                nc.vector.reciprocal(recip[:], den[:])
                cs = ph2.tile([1, 256], F32R, name="cs", tag="cs", bufs=2)
                nc.vector.tensor_copy(cs[:, 0:128], recip[:, 0:128])
                nc.vector.tensor_mul(
                    cs[:, 128:256], recip[:, 128:256],
                    lam0_[:, qt * 128:(qt + 1) * 128],
                )
                b_ps = psum.tile([128, 256], F32, name="b_ps", tag="s2", bufs=3)
                nc.tensor.matmul(
                    b_ps[:], ones_sb[0:1, :], cs[0:1, :], start=True, stop=True,
                )
                b_sb = ph2.tile([128, 256], F32, name="b_sb", tag="bsb", bufs=2)
                nc.scalar.copy(b_sb[:], b_ps[:])
                t0 = ph2.tile([128, 128], F32, name="t0", tag="t0", bufs=2)
                nc.vector.tensor_mul(t0[:], ctx_ps[:, 0:128], b_sb[:, 0:128])
                t1b = ph2.tile([128, 128], F32, name="t1b", tag="t1b", bufs=2)
                nc.vector.tensor_mul(t1b[:], ctx_ps[:, 128:256], b_sb[:, 128:256])
                nc.vector.tensor_sub(
                    diffT[:, p_, qt * 128:(qt + 1) * 128], t0[:], t1b[:]
                )

            for p in range(NKV):
                qt_sb = ph2.tile([128, 2, L], F32R, name="qt_sb", tag="qt", bufs=2)
                nc.sync.dma_start(
                    qt_sb[:], qkT_d[2 * p:2 * p + 2, :, :].rearrange("h p l -> p h l")
                )
                lam0 = ph2.tile([1, L], F32, name="lam0", tag="lam0", bufs=2)
                nc.sync.dma_start(lam0[:], lam_sb[p:p + 1, :])
                kt_sb = ph2.tile([128, L], F32R, name="kt_sb", tag="kt", bufs=2)
                nc.sync.dma_start(kt_sb[:], qkT_d[NQ + p, :, :])
                vp_sb = ph2.tile([128, LT, 128], F32R, name="vp_sb", tag="vp", bufs=2)
                nc.sync.dma_start(vp_sb[:], v_r[:, :, p * 128:(p + 1) * 128])

                for qt in range(LT):
                    ctx_ps = psum.tile([128, 256], F32, name="ctx_ps", tag="ctx", bufs=2)
                    rs_ps = psum.tile([1, 512], F32, name="rs_ps", tag="small", bufs=1)
                    pend = []  # block states awaiting ctx/rs emission
                    for b in range((qt + 2) // 2):
                        kcs = [kc for kc in (2 * b, 2 * b + 1) if kc <= qt]
                        wid = 256 * len(kcs)
                        s_ps = psum.tile([128, 512], F32, name="s_ps", tag="s2", bufs=3)
                        for j, kc in enumerate(kcs):
                            nc.tensor.matmul(
                                s_ps[:, j * 256:(j + 1) * 256],
                                kt_sb[:, kc * 128:(kc + 1) * 128],
                                qt_sb[:, :, qt * 128:(qt + 1) * 128],
                                start=True, stop=True, skip_group_check=True,
                            )
                        if b == 0:
                            emit_norm()  # previous qtile's tail rides under S
                        if len(pend) >= 2:
                            emit_block(pend.pop(0))
                        e_sb = ph2.tile([128, 512], F32R, name="e_sb", tag="e", bufs=3)
                        nc.scalar.activation(
                            e_sb[:, 0:wid], s_ps[:, 0:wid],
                            mybir.ActivationFunctionType.Exp, scale=SCALE,
                        )
                        if kcs[-1] == qt:
                            nc.vector.tensor_mul(
                                e_sb[:, (len(kcs) - 1) * 256:wid],
                                e_sb[:, (len(kcs) - 1) * 256:wid], mask_sb[:],
                            )
                        pend.append((ctx_ps, rs_ps, e_sb, kcs, qt, wid))
                    for st in pend:
                        emit_block(st)
                    pend_norm[0] = (ctx_ps, rs_ps, qt, p, lam0)
            emit_norm()

        # ================= Phase 3: output projection =================
        with tc.tile_pool(name="ph3", bufs=1) as ph3:
            wo_sb = ph3.tile([128, NKV, D], F32R)
            nc.sync.dma_start(wo_sb[:], Wo.rearrange("(p d) o -> d p o", d=128))
            for ot in range(LT):
                for qch in range(LCH):
                    o_ps = psum.tile([128, 512], F32, name="o_ps", tag="mm512", bufs=2)
                    for p in range(NKV):
                        nc.tensor.matmul(
                            o_ps[:],
                            wo_sb[:, p, ot * 128:(ot + 1) * 128],
                            diffT[:, p, qch * 512:(qch + 1) * 512],
                            start=(p == 0), stop=(p == NKV - 1),
                        )
                    o_sb = ph3.tile([128, 512], F32, name="o_sb", tag="osb", bufs=3)
                    nc.scalar.copy(o_sb[:], o_ps[:])
                    nc.sync.dma_start(
                        outT[ot * 128:(ot + 1) * 128, qch * 512:(qch + 1) * 512], o_sb[:]
                    )

    nc.finalize()
    return nc


def _host_tables():
    half = DH // 2
    inv_freq = 1.0 / (ROPE_BASE ** (np.arange(0, half, dtype=np.float64) * 2.0 / DH))
    freqs = np.arange(L, dtype=np.float64)[:, None] * inv_freq[None, :]  # [L, half]
    emb = np.concatenate([freqs, freqs], axis=-1)  # [L, DH]
    cosT = np.ascontiguousarray(np.cos(emb).T.astype(np.float32))  # [DH, L]
    sinT = np.sin(emb).T.astype(np.float32)
    sinTs = np.concatenate([-sinT[:half], sinT[half:]], axis=0)
    sinTs = np.ascontiguousarray(sinTs.astype(np.float32))
    tri = np.triu(np.ones((128, 128), dtype=np.float32))  # keep k' <= q'
    maskT = np.ascontiguousarray(np.concatenate([tri, tri], axis=1))
    ones = np.ones((128, 128), dtype=np.float32)
    return cosT, sinTs, maskT, ones


_NC_CACHE = []


def kernel(x, Wq, Wk, Wv, Wl, bl, Wo):
    x = np.asarray(x, dtype=np.float32)
    Wq = np.asarray(Wq, dtype=np.float32)
    Wk = np.asarray(Wk, dtype=np.float32)
    Wv = np.asarray(Wv, dtype=np.float32)
    Wl = np.asarray(Wl, dtype=np.float32)
    bl = np.asarray(bl, dtype=np.float32)
    Wo = np.asarray(Wo, dtype=np.float32)

    cosT, sinTs, maskT, ones = _host_tables()
    Wq3 = Wq.reshape(D, 2 * NH, DH)
    Wk3 = Wk.reshape(D, NH, DH)

    in_maps = []
    for c in range(8):
        b, g = divmod(c, G)
        wq_s = Wq3[:, 8 * g:8 * g + NQ, :].reshape(D, NQ * DH)
        wk_s = Wk3[:, G * g:G * g + NKV, :].reshape(D, NKV * DH)
        in_maps.append({
            "xT": np.ascontiguousarray(x[b].T),
            "Wqk": np.ascontiguousarray(np.concatenate([wq_s, wk_s], axis=1)),
            "Wv": np.ascontiguousarray(Wv[:, DH * G * g:DH * G * g + NKV * DH]),
            "Wl": np.ascontiguousarray(Wl[:, G * g:G * g + NKV]),
            "blv": np.ascontiguousarray(bl[G * g:G * g + NKV].reshape(NKV, 1)),
            "Wo": np.ascontiguousarray(Wo[512 * g:512 * (g + 1), :]),
            "cosT": cosT,
            "sinTs": sinTs,
            "maskT": maskT,
            "onesin": ones,
        })

    if not _NC_CACHE:
        _NC_CACHE.append(build_kernel())
    nc = _NC_CACHE[0]
    res = run_bass_kernel_spmd(nc, in_maps, core_ids=list(range(8)))

    out = np.empty((B, L, D), dtype=np.float32)
    for b in range(B):
        acc = res.results[4 * b]["outT"].copy()
        for g in range(1, G):
            acc += res.results[4 * b + g]["outT"]
        out[b] = acc.T
    return out


# revision 8
# speedup vs baseline: 1.3025x; 1.2213x over previous
"""DifferentialCausalAttention on 8 Trainium2 NeuronCores.

Sharding: 8 cores = 2 batches x 4 head-groups (tensor-parallel over heads).
Core c handles batch b = c // 4 and head-group g = c % 4:
  - query heads 8g..8g+7 (4 pairs), kv heads 4g..4g+3, lambda cols 4g..4g+3
  - W_O rows 512g..512g+511 -> partial output, host-summed over the 4 groups.

All matmuls run in float32r (full-rate fp32 mode on the PE).
Layouts on device: Q^T/K^T as [dh, L] (dh on partitions), V as [L, d],
attention computed transposed (S^T = [k, q]) so no P-transposes are needed.
"""
from contextlib import ExitStack

import numpy as np

import concourse.bass as bass
import concourse.mybir as mybir
import concourse.tile as tile
from concourse import bacc
from concourse.bass_utils import run_bass_kernel_spmd

F32 = mybir.dt.float32
F32R = mybir.dt.float32r

B, L, D, NH = 2, 2048, 2048, 16
DH = D // NH            # 128
G = 4                   # head groups (cores per batch)
NKV = NH // G           # kv heads per core = 4
NQ = 2 * NKV            # query heads per core = 8
CQK = NQ * DH + NKV * DH  # 1536 projection cols (Q then K)
CT = CQK // 128         # 12 column tiles (0-7 Q heads, 8-11 K heads)
DC = D // 128           # 16 contraction chunks
LCH = L // 512          # 4 L-chunks
LT = L // 128           # 16 L-tiles / q-tiles
SCALE = 1.0 / float(np.sqrt(DH))
ROPE_BASE = 10000.0


def build_kernel() -> bacc.Bacc:
    nc = bacc.Bacc("TRN2", target_bir_lowering=False, debug=False)

    xT = nc.dram_tensor("xT", [D, L], F32R, kind="ExternalInput")
    Wqk = nc.dram_tensor("Wqk", [D, CQK], F32R, kind="ExternalInput")
    Wv = nc.dram_tensor("Wv", [D, NKV * DH], F32R, kind="ExternalInput")
    Wl = nc.dram_tensor("Wl", [D, NKV], F32R, kind="ExternalInput")
    blv = nc.dram_tensor("blv", [NKV, 1], F32, kind="ExternalInput")
    Wo = nc.dram_tensor("Wo", [NKV * DH, D], F32R, kind="ExternalInput")
    cosT = nc.dram_tensor("cosT", [DH, L], F32, kind="ExternalInput")
    sinTs = nc.dram_tensor("sinTs", [DH, L], F32, kind="ExternalInput")
    maskT = nc.dram_tensor("maskT", [128, 256], F32R, kind="ExternalInput")
    onesin = nc.dram_tensor("onesin", [128, 128], F32R, kind="ExternalInput")
    outT = nc.dram_tensor("outT", [D, L], F32, kind="ExternalOutput")

    with ExitStack() as ctx:
        tc = ctx.enter_context(tile.TileContext(nc))

        persist = ctx.enter_context(tc.tile_pool(name="persist", bufs=1))
        dram = ctx.enter_context(tc.tile_pool(name="dram", bufs=1, space="DRAM"))
        psum = ctx.enter_context(tc.tile_pool(name="psum", bufs=1, space="PSUM"))

        # ---- persistent tiles ----
        mask_sb = persist.tile([128, 256], F32R)
        nc.sync.dma_start(mask_sb[:], maskT[:, :])
        ones_sb = persist.tile([128, 128], F32R)
        nc.sync.dma_start(ones_sb[:], onesin[:, :])
        bl_sb = persist.tile([NKV, 1], F32)
        nc.sync.dma_start(bl_sb[:], blv[:, :])
        lam_sb = persist.tile([NKV, L], F32)          # sigmoid(x@Wl+bl), row per kv head
        diffT = persist.tile([128, NKV, L], F32R)     # (ctx0 - lam*ctx1)^T per head

        # DRAM scratch between phases
        qkT_d = dram.tile([CT, 128, L], F32R)         # Q^T/K^T after RoPE
        v_d = dram.tile([L, NKV * DH], F32R)          # V in [L, d] layout

        # ================= Phase 1: projections + RoPE =================
        with tc.tile_pool(name="ph1", bufs=1) as ph1:
            wv_sb = ph1.tile([128, DC, NKV * DH], F32R)
            nc.sync.dma_start(wv_sb[:], Wv.rearrange("(dc p) c -> p dc c", p=128))
            wl_sb = ph1.tile([128, DC, NKV], F32R)
            nc.sync.dma_start(wl_sb[:], Wl.rearrange("(dc p) c -> p dc c", p=128))
            cos_sb = ph1.tile([128, L], F32)
            nc.sync.dma_start(cos_sb[:], cosT[:, :])
            sin_sb = ph1.tile([128, L], F32)
            nc.sync.dma_start(sin_sb[:], sinTs[:, :])

            xTr = xT.rearrange("(dc p) l -> p dc l", p=128)
            wqkr = Wqk.rearrange("(dc p) c -> p dc c", p=128)

            for lch in range(LCH):
                ls = slice(lch * 512, (lch + 1) * 512)
                xs = ph1.tile([128, DC, 512], F32R, name="xs", tag="xs", bufs=2)
                nc.sync.dma_start(xs[:], xTr[:, :, ls])

                # --- Q^T / K^T column tiles + RoPE ---
                for ct in range(CT):
                    wt = ph1.tile([128, DC, 128], F32R, name="wt", tag="wt", bufs=3)
                    nc.sync.dma_start(wt[:], wqkr[:, :, ct * 128:(ct + 1) * 128])
                    qk_ps = psum.tile([128, 512], F32, name="qk_ps", tag="mm512", bufs=2)
                    for dc in range(DC):
                        nc.tensor.matmul(
                            qk_ps[:], wt[:, dc, :], xs[:, dc, :],
                            start=(dc == 0), stop=(dc == DC - 1),
                        )
                    # RoPE: qr = qk*cos + rot(qk)*sin_signed
                    rot = ph1.tile([128, 512], F32, name="rot", tag="rot", bufs=2)
                    nc.scalar.copy(rot[0:64, :], qk_ps[64:128, :])
                    nc.scalar.copy(rot[64:128, :], qk_ps[0:64, :])
                    t1 = ph1.tile([128, 512], F32, name="t1", tag="t1", bufs=2)
                    nc.vector.tensor_mul(t1[:], qk_ps[:], cos_sb[:, ls])
                    t2 = ph1.tile([128, 512], F32, name="t2", tag="t2", bufs=2)
                    nc.vector.tensor_mul(t2[:], rot[:], sin_sb[:, ls])
                    qkr_sb = ph1.tile([128, 512], F32R, name="qkr_sb", tag="qkr", bufs=3)
                    nc.vector.tensor_add(qkr_sb[:], t1[:], t2[:])
                    nc.sync.dma_start(qkT_d[ct, :, ls], qkr_sb[:])

                # --- V tiles ---
                for lt in range(4):
                    v_ps = psum.tile([128, 512], F32, name="v_ps", tag="mm512", bufs=2)
                    for dc in range(DC):
                        nc.tensor.matmul(
                            v_ps[:], xs[:, dc, lt * 128:(lt + 1) * 128], wv_sb[:, dc, :],
                            start=(dc == 0), stop=(dc == DC - 1),
                        )
                    v_sb = ph1.tile([128, 512], F32R, name="v_sb", tag="v_sb", bufs=2)
                    nc.scalar.copy(v_sb[:], v_ps[:])
                    nc.sync.dma_start(
                        v_d[lch * 512 + lt * 128: lch * 512 + (lt + 1) * 128, :], v_sb[:]
                    )

                # --- lambda ---
                lam_ps = psum.tile([NKV, 512], F32, name="lam_ps", tag="small", bufs=1)
                for dc in range(DC):
                    nc.tensor.matmul(
                        lam_ps[:], wl_sb[:, dc, :], xs[:, dc, :],
                        start=(dc == 0), stop=(dc == DC - 1),
                    )
                nc.scalar.activation(
                    lam_sb[:, ls], lam_ps[:],
                    mybir.ActivationFunctionType.Sigmoid, bias=bl_sb[:, 0:1],
                )

        # ================= Phase 2: causal attention per head pair =================
        # Two q-tiles (a "superblock": A=2sb, B=2sb+1) are processed at once so
        # every moving operand is 512 wide: columns ordered (qtile, head, l) =
        # [A.h0 | A.h1 | B.h0 | B.h1]. Per k-chunk: one S matmul [128,512], one
        # exp, one ctx matmul, one rowsum matmul. Chunk kc==A is full width but
        # masks its [0:256] half; chunk kc==B covers only [256:512]. ctx/rs for
        # chunk kc are emitted after the S matmul of chunk kc+2 (PE never waits
        # on ACT); the normalization tail is deferred into the next superblock.
        with tc.tile_pool(name="ph2", bufs=1) as ph2:
            v_r = v_d.rearrange("(kc pp) d -> pp kc d", pp=128)
            pend_norm = [None]

            def emit_block(st):
                ctx_ps, rs_ps, e_sb, kc, qtB, off, wid = st
                nc.tensor.matmul(
                    ctx_ps[:, off:off + wid], vp_sb[:, kc, :], e_sb[:, off:off + wid],
                    start=(kc == 0), stop=(kc == qtB), skip_group_check=True,
                )
                nc.tensor.matmul(
                    rs_ps[0:1, off:off + wid], ones_sb[:, 0:1], e_sb[:, off:off + wid],
                    start=(kc == 0), stop=(kc == qtB), skip_group_check=True,
                )

            def emit_norm():
                st = pend_norm[0]
                if st is None:
                    return
                pend_norm[0] = None
                ctx_ps, rs_ps, qtA, p_, lam0_ = st
                rs_sb = ph2.tile([1, 512], F32, name="rs_sb", tag="rs_sb", bufs=2)
                nc.scalar.copy(rs_sb[:], rs_ps[:])
                recip = ph2.tile([1, 2, 256], F32, name="recip", tag="recip", bufs=2)
                nc.vector.reciprocal_approx_fast(
                    recip.rearrange("p t l -> p (t l)"), rs_sb[:]
                )
                cs = ph2.tile([1, 2, 256], F32R, name="cs", tag="cs", bufs=2)
                nc.vector.tensor_copy(cs[:, :, 0:128], recip[:, :, 0:128])
                nc.vector.tensor_mul(
                    cs[:, :, 128:256], recip[:, :, 128:256],
                    lam0_[:, qtA * 128:(qtA + 2) * 128].rearrange(
                        "p (t l) -> p t l", t=2
                    ),
                )
                b_ps = psum.tile([128, 512], F32, name="b_ps", tag="s2", bufs=3)
                nc.tensor.matmul(
                    b_ps[:], ones_sb[0:1, :], cs.rearrange("p t l -> p (t l)"),
                    start=True, stop=True,
                )
                b_sb = ph2.tile([128, 2, 256], F32, name="b_sb", tag="bsb", bufs=2)
                nc.scalar.copy(b_sb.rearrange("p t l -> p (t l)"), b_ps[:])
                ctx3 = ctx_ps.rearrange("p (t l) -> p t l", t=2)
                t0 = ph2.tile([128, 2, 128], F32, name="t0", tag="t0", bufs=2)
                nc.vector.tensor_mul(t0[:], ctx3[:, :, 0:128], b_sb[:, :, 0:128])
                t1b = ph2.tile([128, 2, 128], F32, name="t1b", tag="t1b", bufs=2)
                nc.vector.tensor_mul(t1b[:], ctx3[:, :, 128:256], b_sb[:, :, 128:256])
                nc.vector.tensor_sub(
                    diffT[:, p_, qtA * 128:(qtA + 2) * 128],
                    t0.rearrange("p t l -> p (t l)"),
                    t1b.rearrange("p t l -> p (t l)"),
                )

            for p in range(NKV):
                # [128, 16(t), 2(h), 128(l)] so superblock slices are contiguous
                qt_sb = ph2.tile([128, LT, 2, 128], F32R, name="qt_sb", tag="qt", bufs=2)
                nc.sync.dma_start(
                    qt_sb[:],
                    qkT_d[2 * p:2 * p + 2, :, :].rearrange(
                        "h p (t l) -> p t h l", t=LT
                    ),
                )
                lam0 = ph2.tile([1, L], F32, name="lam0", tag="lam0", bufs=2)
                nc.sync.dma_start(lam0[:], lam_sb[p:p + 1, :])
                kt_sb = ph2.tile([128, L], F32R, name="kt_sb", tag="kt", bufs=2)
                nc.sync.dma_start(kt_sb[:], qkT_d[NQ + p, :, :])
                vp_sb = ph2.tile([128, LT, 128], F32R, name="vp_sb", tag="vp", bufs=2)
                nc.sync.dma_start(vp_sb[:], v_r[:, :, p * 128:(p + 1) * 128])

                for sb in range(LT // 2):
                    qtA, qtB = 2 * sb, 2 * sb + 1
                    ctx_ps = psum.tile([128, 512], F32, name="ctx_ps", tag="ctx", bufs=2)
                    rs_ps = psum.tile([1, 512], F32, name="rs_ps", tag="small", bufs=1)
                    pend = []
                    for kc in range(qtB + 1):
                        off, wid = (256, 256) if kc == qtB else (0, 512)
                        s_ps = psum.tile([128, 512], F32, name="s_ps", tag="s2", bufs=3)
                        if wid == 512:
                            rhs = qt_sb[:, qtA:qtA + 2, :, :]
                        else:
                            rhs = qt_sb[:, qtB, :, :]
                        nc.tensor.matmul(
                            s_ps[:, off:off + wid],
                            kt_sb[:, kc * 128:(kc + 1) * 128],
                            rhs,
                            start=True, stop=True, skip_group_check=True,
                        )
                        if kc == 0:
                            emit_norm()  # previous superblock's tail
                        if len(pend) >= 2:
                            emit_block(pend.pop(0))
                        e_sb = ph2.tile([128, 512], F32R, name="e_sb", tag="e", bufs=3)
                        nc.scalar.activation(
                            e_sb[:, off:off + wid], s_ps[:, off:off + wid],
                            mybir.ActivationFunctionType.Exp, scale=SCALE,
                        )
                        if kc == qtA:
                            nc.vector.tensor_mul(
                                e_sb[:, 0:256], e_sb[:, 0:256], mask_sb[:]
                            )
                        elif kc == qtB:
                            nc.vector.tensor_mul(
                                e_sb[:, 256:512], e_sb[:, 256:512], mask_sb[:]
                            )
                        pend.append((ctx_ps, rs_ps, e_sb, kc, qtB, off, wid))
                    for st in pend:
                        emit_block(st)
                    pend_norm[0] = (ctx_ps, rs_ps, qtA, p, lam0)
            emit_norm()

        # ================= Phase 3: output projection =================
        with tc.tile_pool(name="ph3", bufs=1) as ph3:
            wo_sb = ph3.tile([128, NKV, D], F32R)
            nc.sync.dma_start(wo_sb[:], Wo.rearrange("(p d) o -> d p o", d=128))
            for ot in range(LT):
                for qch in range(LCH):
                    o_ps = psum.tile([128, 512], F32, name="o_ps", tag="mm512", bufs=2)
                    for p in range(NKV):
                        nc.tensor.matmul(
                            o_ps[:],
                            wo_sb[:, p, ot * 128:(ot + 1) * 128],
                            diffT[:, p, qch * 512:(qch + 1) * 512],
                            start=(p == 0), stop=(p == NKV - 1),
                        )
                    o_sb = ph3.tile([128, 512], F32, name="o_sb", tag="osb", bufs=3)
                    nc.scalar.copy(o_sb[:], o_ps[:])
                    nc.sync.dma_start(
                        outT[ot * 128:(ot + 1) * 128, qch * 512:(qch + 1) * 512], o_sb[:]
                    )

    nc.finalize()
    return nc


def _host_tables():
    half = DH // 2
    inv_freq = 1.0 / (ROPE_BASE ** (np.arange(0, half, dtype=np.float64) * 2.0 / DH))
    freqs = np.arange(L, dtype=np.float64)[:, None] * inv_freq[None, :]  # [L, half]
    emb = np.concatenate([freqs, freqs], axis=-1)  # [L, DH]
    cosT = np.ascontiguousarray(np.cos(emb).T.astype(np.float32))  # [DH, L]
    sinT = np.sin(emb).T.astype(np.float32)
    sinTs = np.concatenate([-sinT[:half], sinT[half:]], axis=0)
    sinTs = np.ascontiguousarray(sinTs.astype(np.float32))
    tri = np.triu(np.ones((128, 128), dtype=np.float32))  # keep k' <= q'
    maskT = np.ascontiguousarray(np.concatenate([tri, tri], axis=1))
    ones = np.ones((128, 128), dtype=np.float32)
    return cosT, sinTs, maskT, ones


_NC_CACHE = []


def kernel(x, Wq, Wk, Wv, Wl, bl, Wo):
    x = np.asarray(x, dtype=np.float32)
    Wq = np.asarray(Wq, dtype=np.float32)
    Wk = np.asarray(Wk, dtype=np.float32)
    Wv = np.asarray(Wv, dtype=np.float32)
    Wl = np.asarray(Wl, dtype=np.float32)
    bl = np.asarray(bl, dtype=np.float32)
    Wo = np.asarray(Wo, dtype=np.float32)

    cosT, sinTs, maskT, ones = _host_tables()
    Wq3 = Wq.reshape(D, 2 * NH, DH)
    Wk3 = Wk.reshape(D, NH, DH)

    in_maps = []
    for c in range(8):
        b, g = divmod(c, G)
        wq_s = Wq3[:, 8 * g:8 * g + NQ, :].reshape(D, NQ * DH)
        wk_s = Wk3[:, G * g:G * g + NKV, :].reshape(D, NKV * DH)
        in_maps.append({
            "xT": np.ascontiguousarray(x[b].T),
            "Wqk": np.ascontiguousarray(np.concatenate([wq_s, wk_s], axis=1)),
            "Wv": np.ascontiguousarray(Wv[:, DH * G * g:DH * G * g + NKV * DH]),
            "Wl": np.ascontiguousarray(Wl[:, G * g:G * g + NKV]),
            "blv": np.ascontiguousarray(bl[G * g:G * g + NKV].reshape(NKV, 1)),
            "Wo": np.ascontiguousarray(Wo[512 * g:512 * (g + 1), :]),
            "cosT": cosT,
            "sinTs": sinTs,
            "maskT": maskT,
            "onesin": ones,
        })

    if not _NC_CACHE:
        _NC_CACHE.append(build_kernel())
    nc = _NC_CACHE[0]
    res = run_bass_kernel_spmd(nc, in_maps, core_ids=list(range(8)))

    out = np.empty((B, L, D), dtype=np.float32)
    for b in range(B):
        acc = res.results[4 * b]["outT"].copy()
        for g in range(1, G):
            acc += res.results[4 * b + g]["outT"]
        out[b] = acc.T
    return out


# revision 10
# speedup vs baseline: 1.5910x; 1.2215x over previous
"""DifferentialCausalAttention on 8 Trainium2 NeuronCores.

Sharding: 8 cores = 2 batches x 4 head-groups (tensor-parallel over heads).
Core c handles batch b = c // 4 and head-group g = c % 4:
  - query heads 8g..8g+7 (4 pairs), kv heads 4g..4g+3, lambda cols 4g..4g+3
  - W_O rows 512g..512g+511 -> partial output, host-summed over the 4 groups.

All matmuls run in float32r (full-rate fp32 mode on the PE).
Layouts on device: Q^T/K^T as [dh, L] (dh on partitions), V as [L, d],
attention computed transposed (S^T = [k, q]) so no P-transposes are needed.
"""
from contextlib import ExitStack

import numpy as np

import concourse.bass as bass
import concourse.mybir as mybir
import concourse.tile as tile
from concourse import bacc
from concourse.bass_utils import run_bass_kernel_spmd

F32 = mybir.dt.float32
F32R = mybir.dt.float32r

B, L, D, NH = 2, 2048, 2048, 16
DH = D // NH            # 128
G = 4                   # head groups (cores per batch)
NKV = NH // G           # kv heads per core = 4
NQ = 2 * NKV            # query heads per core = 8
CQK = NQ * DH + NKV * DH  # 1536 projection cols (Q then K)
CT = CQK // 128         # 12 column tiles (0-7 Q heads, 8-11 K heads)
DC = D // 128           # 16 contraction chunks
LCH = L // 512          # 4 L-chunks
LT = L // 128           # 16 L-tiles / q-tiles
SCALE = 1.0 / float(np.sqrt(DH))
ROPE_BASE = 10000.0


def build_kernel() -> bacc.Bacc:
    nc = bacc.Bacc("TRN2", target_bir_lowering=False, debug=False)

    xT = nc.dram_tensor("xT", [D, L], F32R, kind="ExternalInput")
    Wqk = nc.dram_tensor("Wqk", [D, CQK], F32R, kind="ExternalInput")
    Wv = nc.dram_tensor("Wv", [D, NKV * DH], F32R, kind="ExternalInput")
    Wl = nc.dram_tensor("Wl", [D, NKV], F32R, kind="ExternalInput")
    blv = nc.dram_tensor("blv", [NKV, 1], F32, kind="ExternalInput")
    Wo = nc.dram_tensor("Wo", [NKV * DH, D], F32R, kind="ExternalInput")
    cosT = nc.dram_tensor("cosT", [DH, L], F32, kind="ExternalInput")
    sinTs = nc.dram_tensor("sinTs", [DH, L], F32, kind="ExternalInput")
    maskT = nc.dram_tensor("maskT", [128, 256], F32R, kind="ExternalInput")
    onesin = nc.dram_tensor("onesin", [128, 128], F32R, kind="ExternalInput")
    outT = nc.dram_tensor("outT", [D, L], F32, kind="ExternalOutput")

    with ExitStack() as ctx:
        tc = ctx.enter_context(tile.TileContext(nc))

        persist = ctx.enter_context(tc.tile_pool(name="persist", bufs=1))
        dram = ctx.enter_context(tc.tile_pool(name="dram", bufs=1, space="DRAM"))

        # ---- persistent tiles ----
        mask_sb = persist.tile([128, 256], F32R)
        nc.sync.dma_start(mask_sb[:], maskT[:, :])
        ones_sb = persist.tile([128, 128], F32R)
        nc.sync.dma_start(ones_sb[:], onesin[:, :])
        bl_sb = persist.tile([NKV, 1], F32)
        nc.sync.dma_start(bl_sb[:], blv[:, :])
        lam_sb = persist.tile([NKV, L], F32)          # sigmoid(x@Wl+bl), row per kv head
        diffT = persist.tile([128, NKV, L], F32R)     # (ctx0 - lam*ctx1)^T per head

        # DRAM scratch between phases
        qkT_d = dram.tile([CT, 128, L], F32R)         # Q^T/K^T after RoPE
        v_d = dram.tile([L, NKV * DH], F32R)          # V in [L, d] layout

        # ================= Phase 1: projections + RoPE =================
        with tc.tile_pool(name="ph1", bufs=1) as ph1, \
                tc.tile_pool(name="ps1", bufs=1, space="PSUM") as ps1:
            xTr = xT.rearrange("(dc p) l -> p dc l", p=128)
            wqkr = Wqk.rearrange("(dc p) c -> p dc c", p=128)
            wv_sb = wl_sb = None

            for lch in range(LCH):
                ls = slice(lch * 512, (lch + 1) * 512)
                xs = ph1.tile([128, DC, 512], F32R, name="xs", tag="xs", bufs=2)
                nc.sync.dma_start(xs[:], xTr[:, :, ls])
                cos_sb = ph1.tile([128, 512], F32, name="cos_sb", tag="cos", bufs=2)
                nc.sync.dma_start(cos_sb[:], cosT[:, ls])
                sin_sb = ph1.tile([128, 512], F32, name="sin_sb", tag="sin", bufs=2)
                nc.sync.dma_start(sin_sb[:], sinTs[:, ls])

                # --- Q^T / K^T column tiles + RoPE ---
                for ct in range(CT):
                    wt = ph1.tile([128, DC, 128], F32R, name="wt", tag="wt", bufs=3)
                    nc.sync.dma_start(wt[:], wqkr[:, :, ct * 128:(ct + 1) * 128])
                    qk_ps = ps1.tile([128, 512], F32, name="qk_ps", tag="mm512", bufs=4)
                    for dc in range(DC):
                        nc.tensor.matmul(
                            qk_ps[:], wt[:, dc, :], xs[:, dc, :],
                            start=(dc == 0), stop=(dc == DC - 1),
                        )
                    # RoPE: qr = qk*cos + rot(qk)*sin_signed
                    rot = ph1.tile([128, 512], F32, name="rot", tag="rot", bufs=2)
                    nc.scalar.copy(rot[0:64, :], qk_ps[64:128, :])
                    nc.scalar.copy(rot[64:128, :], qk_ps[0:64, :])
                    t1 = ph1.tile([128, 512], F32, name="t1", tag="t1", bufs=2)
                    nc.vector.tensor_mul(t1[:], qk_ps[:], cos_sb[:])
                    t2 = ph1.tile([128, 512], F32, name="t2", tag="t2", bufs=2)
                    nc.vector.tensor_mul(t2[:], rot[:], sin_sb[:])
                    qkr_sb = ph1.tile([128, 512], F32R, name="qkr_sb", tag="qkr", bufs=3)
                    nc.vector.tensor_add(qkr_sb[:], t1[:], t2[:])
                    nc.sync.dma_start(qkT_d[ct, :, ls], qkr_sb[:])
                    if lch == 0 and ct == 0:
                        # emit the big constant loads after the first column
                        # tile is underway so they don't delay the first matmul
                        wv_sb = ph1.tile([128, DC, NKV * DH], F32R)
                        nc.sync.dma_start(
                            wv_sb[:], Wv.rearrange("(dc p) c -> p dc c", p=128)
                        )
                        wl_sb = ph1.tile([128, DC, NKV], F32R)
                        nc.sync.dma_start(
                            wl_sb[:], Wl.rearrange("(dc p) c -> p dc c", p=128)
                        )

                # --- V tiles ---
                for lt in range(4):
                    v_ps = ps1.tile([128, 512], F32, name="v_ps", tag="mm512", bufs=4)
                    for dc in range(DC):
                        nc.tensor.matmul(
                            v_ps[:], xs[:, dc, lt * 128:(lt + 1) * 128], wv_sb[:, dc, :],
                            start=(dc == 0), stop=(dc == DC - 1),
                        )
                    v_sb = ph1.tile([128, 512], F32R, name="v_sb", tag="v_sb", bufs=2)
                    nc.scalar.copy(v_sb[:], v_ps[:])
                    nc.sync.dma_start(
                        v_d[lch * 512 + lt * 128: lch * 512 + (lt + 1) * 128, :], v_sb[:]
                    )

                # --- lambda ---
                lam_ps = ps1.tile([NKV, 512], F32, name="lam_ps", tag="small", bufs=2)
                for dc in range(DC):
                    nc.tensor.matmul(
                        lam_ps[:], wl_sb[:, dc, :], xs[:, dc, :],
                        start=(dc == 0), stop=(dc == DC - 1),
                    )
                nc.scalar.activation(
                    lam_sb[:, ls], lam_ps[:],
                    mybir.ActivationFunctionType.Sigmoid, bias=bl_sb[:, 0:1],
                )

        # ================= Phase 2: causal attention per head pair =================
        # Two q-tiles (a "superblock": A=2sb, B=2sb+1) are processed at once so
        # every moving operand is 512 wide: columns ordered (qtile, head, l) =
        # [A.h0 | A.h1 | B.h0 | B.h1]. Per k-chunk: one S matmul [128,512], one
        # exp, one ctx matmul, one rowsum matmul. Chunk kc==A is full width but
        # masks its [0:256] half; chunk kc==B covers only [256:512]. ctx/rs for
        # chunk kc are emitted after the S matmul of chunk kc+2 (PE never waits
        # on ACT); the normalization tail is deferred into the next superblock.
        with tc.tile_pool(name="ph2", bufs=1) as ph2, \
                tc.tile_pool(name="ps2", bufs=1, space="PSUM") as ps2:
            v_r = v_d.rearrange("(kc pp) d -> pp kc d", pp=128)
            pend_norm = []

            def emit_block(st):
                ctx_ps, rs_ps, e_sb, kc, qtB, off, wid = st
                nc.tensor.matmul(
                    ctx_ps[:, off:off + wid], vp_sb[:, kc, :], e_sb[:, off:off + wid],
                    start=(kc == 0), stop=(kc == qtB), skip_group_check=True,
                )
                nc.tensor.matmul(
                    rs_ps[0:1, off:off + wid], ones_sb[:, 0:1], e_sb[:, off:off + wid],
                    start=(kc == 0), stop=(kc == qtB), skip_group_check=True,
                )

            def emit_norm(st):
                ctx_ps, rs_sb, qtA, p_, lam0_ = st
                recip = ph2.tile([1, 2, 256], F32, name="recip", tag="recip", bufs=2)
                nc.vector.reciprocal_approx_fast(
                    recip.rearrange("p t l -> p (t l)"), rs_sb[:]
                )
                cs = ph2.tile([1, 2, 256], F32R, name="cs", tag="cs", bufs=2)
                nc.vector.tensor_copy(cs[:, :, 0:128], recip[:, :, 0:128])
                nc.vector.tensor_mul(
                    cs[:, :, 128:256], recip[:, :, 128:256],
                    lam0_[:, qtA * 128:(qtA + 2) * 128].rearrange(
                        "p (t l) -> p t l", t=2
                    ),
                )
                b_ps = ps2.tile([128, 512], F32, name="b_ps", tag="s2", bufs=4)
                nc.tensor.matmul(
                    b_ps[:], ones_sb[0:1, :], cs.rearrange("p t l -> p (t l)"),
                    start=True, stop=True,
                )
                b_sb = ph2.tile([128, 2, 256], F32, name="b_sb", tag="bsb", bufs=2)
                nc.scalar.copy(b_sb.rearrange("p t l -> p (t l)"), b_ps[:])
                ctx3 = ctx_ps.rearrange("p (t l) -> p t l", t=2)
                t0 = ph2.tile([128, 2, 128], F32, name="t0", tag="t0", bufs=2)
                nc.vector.tensor_mul(t0[:], ctx3[:, :, 0:128], b_sb[:, :, 0:128])
                t1b = ph2.tile([128, 2, 128], F32, name="t1b", tag="t1b", bufs=2)
                nc.vector.tensor_mul(t1b[:], ctx3[:, :, 128:256], b_sb[:, :, 128:256])
                nc.vector.tensor_sub(
                    diffT[:, p_, qtA * 128:(qtA + 2) * 128],
                    t0.rearrange("p t l -> p (t l)"),
                    t1b.rearrange("p t l -> p (t l)"),
                )

            for p in range(NKV):
                # [128, 16(t), 2(h), 128(l)] so superblock slices are contiguous
                qt_sb = ph2.tile([128, LT, 2, 128], F32R, name="qt_sb", tag="qt", bufs=2)
                nc.sync.dma_start(
                    qt_sb[:],
                    qkT_d[2 * p:2 * p + 2, :, :].rearrange(
                        "h p (t l) -> p t h l", t=LT
                    ),
                )
                lam0 = ph2.tile([1, L], F32, name="lam0", tag="lam0", bufs=2)
                nc.sync.dma_start(lam0[:], lam_sb[p:p + 1, :])
                kt_sb = ph2.tile([128, L], F32R, name="kt_sb", tag="kt", bufs=2)
                nc.sync.dma_start(kt_sb[:], qkT_d[NQ + p, :, :])
                vp_sb = ph2.tile([128, LT, 128], F32R, name="vp_sb", tag="vp", bufs=2)
                nc.sync.dma_start(vp_sb[:], v_r[:, :, p * 128:(p + 1) * 128])

                for sb in range(LT // 2):
                    qtA, qtB = 2 * sb, 2 * sb + 1
                    ctx_ps = ps2.tile([128, 512], F32, name="ctx_ps", tag="ctx", bufs=3)
                    rs_ps = ps2.tile([1, 512], F32, name="rs_ps", tag="small", bufs=1)
                    pend = []
                    for kc in range(qtB + 1):
                        off, wid = (256, 256) if kc == qtB else (0, 512)
                        s_ps = ps2.tile([128, 512], F32, name="s_ps", tag="s2", bufs=4)
                        if wid == 512:
                            rhs = qt_sb[:, qtA:qtA + 2, :, :]
                        else:
                            rhs = qt_sb[:, qtB, :, :]
                        nc.tensor.matmul(
                            s_ps[:, off:off + wid],
                            kt_sb[:, kc * 128:(kc + 1) * 128],
                            rhs,
                            start=True, stop=True, skip_group_check=True,
                        )
                        if kc == 0 and len(pend_norm) >= 2:
                            emit_norm(pend_norm.pop(0))  # deferred tails
                        if len(pend) >= 3:
                            emit_block(pend.pop(0))
                        e_sb = ph2.tile([128, 512], F32R, name="e_sb", tag="e", bufs=4)
                        nc.scalar.activation(
                            e_sb[:, off:off + wid], s_ps[:, off:off + wid],
                            mybir.ActivationFunctionType.Exp, scale=SCALE,
                        )
                        if kc == qtA:
                            nc.vector.tensor_mul(
                                e_sb[:, 0:256], e_sb[:, 0:256], mask_sb[:]
                            )
                        elif kc == qtB:
                            nc.vector.tensor_mul(
                                e_sb[:, 256:512], e_sb[:, 256:512], mask_sb[:]
                            )
                        pend.append((ctx_ps, rs_ps, e_sb, kc, qtB, off, wid))
                    for st in pend:
                        emit_block(st)
                    # eager rowsum copy frees the PSUM bank promptly
                    rs_sb = ph2.tile([1, 512], F32, name="rs_sb", tag="rs_sb", bufs=3)
                    nc.scalar.copy(rs_sb[:], rs_ps[:])
                    pend_norm.append((ctx_ps, rs_sb, qtA, p, lam0))
            for st in pend_norm:
                emit_norm(st)

        # ================= Phase 3: output projection =================
        with tc.tile_pool(name="ph3", bufs=1) as ph3, \
                tc.tile_pool(name="ps3", bufs=1, space="PSUM") as ps3:
            wo_sb = ph3.tile([128, NKV, D], F32R)
            nc.sync.dma_start(wo_sb[:], Wo.rearrange("(p d) o -> d p o", d=128))
            for ot in range(LT):
                for qch in range(LCH):
                    o_ps = ps3.tile([128, 512], F32, name="o_ps", tag="mm512", bufs=4)
                    for p in range(NKV):
                        nc.tensor.matmul(
                            o_ps[:],
                            wo_sb[:, p, ot * 128:(ot + 1) * 128],
                            diffT[:, p, qch * 512:(qch + 1) * 512],
                            start=(p == 0), stop=(p == NKV - 1),
                        )
                    o_sb = ph3.tile([128, 512], F32, name="o_sb", tag="osb", bufs=4)
                    nc.scalar.copy(o_sb[:], o_ps[:])
                    nc.sync.dma_start(
                        outT[ot * 128:(ot + 1) * 128, qch * 512:(qch + 1) * 512], o_sb[:]
                    )

    nc.finalize()
    return nc


def _host_tables():
    half = DH // 2
    inv_freq = 1.0 / (ROPE_BASE ** (np.arange(0, half, dtype=np.float64) * 2.0 / DH))
    freqs = np.arange(L, dtype=np.float64)[:, None] * inv_freq[None, :]  # [L, half]
    emb = np.concatenate([freqs, freqs], axis=-1)  # [L, DH]
    cosT = np.ascontiguousarray(np.cos(emb).T.astype(np.float32))  # [DH, L]
    sinT = np.sin(emb).T.astype(np.float32)
    sinTs = np.concatenate([-sinT[:half], sinT[half:]], axis=0)
    sinTs = np.ascontiguousarray(sinTs.astype(np.float32))
    tri = np.triu(np.ones((128, 128), dtype=np.float32))  # keep k' <= q'
    maskT = np.ascontiguousarray(np.concatenate([tri, tri], axis=1))
    ones = np.ones((128, 128), dtype=np.float32)
    return cosT, sinTs, maskT, ones


_NC_CACHE = []


def kernel(x, Wq, Wk, Wv, Wl, bl, Wo):
    x = np.asarray(x, dtype=np.float32)
    Wq = np.asarray(Wq, dtype=np.float32)
    Wk = np.asarray(Wk, dtype=np.float32)
    Wv = np.asarray(Wv, dtype=np.float32)
    Wl = np.asarray(Wl, dtype=np.float32)
    bl = np.asarray(bl, dtype=np.float32)
    Wo = np.asarray(Wo, dtype=np.float32)

    cosT, sinTs, maskT, ones = _host_tables()
    Wq3 = Wq.reshape(D, 2 * NH, DH)
    Wk3 = Wk.reshape(D, NH, DH)

    in_maps = []
    for c in range(8):
        b, g = divmod(c, G)
        wq_s = Wq3[:, 8 * g:8 * g + NQ, :].reshape(D, NQ * DH)
        wk_s = Wk3[:, G * g:G * g + NKV, :].reshape(D, NKV * DH)
        in_maps.append({
            "xT": np.ascontiguousarray(x[b].T),
            "Wqk": np.ascontiguousarray(np.concatenate([wq_s, wk_s], axis=1)),
            "Wv": np.ascontiguousarray(Wv[:, DH * G * g:DH * G * g + NKV * DH]),
            "Wl": np.ascontiguousarray(Wl[:, G * g:G * g + NKV]),
            "blv": np.ascontiguousarray(bl[G * g:G * g + NKV].reshape(NKV, 1)),
            "Wo": np.ascontiguousarray(Wo[512 * g:512 * (g + 1), :]),
            "cosT": cosT,
            "sinTs": sinTs,
            "maskT": maskT,
            "onesin": ones,
        })

    if not _NC_CACHE:
        _NC_CACHE.append(build_kernel())
    nc = _NC_CACHE[0]
    res = run_bass_kernel_spmd(nc, in_maps, core_ids=list(range(8)))

    out = np.empty((B, L, D), dtype=np.float32)
    for b in range(B):
        acc = res.results[4 * b]["outT"].copy()
        for g in range(1, G):
            acc += res.results[4 * b + g]["outT"]
        out[b] = acc.T
    return out
